# revision 13
# baseline (speedup 1.0000x reference)
"""Entmax-1.5 (bisection reference) kernel for Trainium2, 8-core data parallel.

The reference runs 50 bisection iterations on tau with bracket
[min(xs)-1, max(xs)=0], xs = x - rowmax(x), z = 0.5*xs,
y = clip(z - tau, 0)^2, constraint = sum(y) - 1, and the update
  tmin = where(constraint < 0, tau, tmin)
  tmax = where(constraint > 0, tau, tmax)
For any row of width N >= 5 the first midpoint tau_1 = (min(xs)-1)/2
satisfies z_i - tau_1 = (xs_i - min(xs) + 1)/2 >= 1/2 for every i, so
constraint >= N/4 - 1 > 0 at tau_1 and at every later (smaller) tau.
Only tmax ever updates, and the f32 halving sequence collapses onto
tmin = min(xs) - 1 within ~30 iterations. Hence the reference equals

    w_i = (0.5*x_i + b)^2,  b = 0.5*rowmax(x) - rowmin(x) + 1
    out = w / (rowsum(w) + 1e-12)

(verified numerically: 5e-7 elementwise relative vs the 50-iter loop).

Kernel per core (512 rows x 32000 cols f32), per 128-row chunk of 4
column tiles (128 x 8000):
  DVE   rowmax + rowmin per tile (tensor_reduce; overlaps the loads)
  ACT   w = Square(0.5x + b) in place with accumulated rowsum -> S
  DVE   r = 1/(S + 1e-12)
  scale w *= r in place (3 tiles ACT copy-with-scale, 1 tile DVE
    tensor_scalar at 2x), store each tile as it completes.
Emission is software-pipelined (chunk c's loads+stats emitted before
chunk c-1's square/scale phase) so each engine's program order matches
the overlapped schedule. One HBM read + one write: DMA-bound.
"""

import numpy as np

N_CORES = 8
ROWS, COLS = 4096, 32000
RPC = ROWS // N_CORES  # rows per core
P = 128  # SBUF partitions
WTILE = 8000  # column tile width
XBUFS = 6  # x-tile slots (each 128 x WTILE f32; SBUF is 224KB/partition)
DVE_SCALE_TILES = 1  # trailing tiles of the scale pass done on DVE

FMAX = 3.4e38


def _build(rows, cols, wtile, xbufs=XBUFS):
    import concourse.bass as bass
    import concourse.tile as tile
    from concourse import bacc, mybir

    f32 = mybir.dt.float32
    AX = mybir.AxisListType.X
    ALU = mybir.AluOpType
    ACTF = mybir.ActivationFunctionType

    assert rows % P == 0 and cols % wtile == 0
    nchunks = rows // P
    ntiles = cols // wtile

    # Bacc (not raw Bass): its compile() runs generate_event_semaphores,
    # which splits multi-wait sync_info to satisfy the TRN2 1-wait/inst limit.
    nc = bacc.Bacc()
    x = nc.declare_dram_parameter("x", [rows, cols], f32, isOutput=False)
    out = nc.declare_dram_parameter("out", [rows, cols], f32, isOutput=True)

    with tile.TileContext(nc) as tc:
        with (
            tc.tile_pool(name="xp", bufs=xbufs) as xp,
            tc.tile_pool(name="sp", bufs=4) as sp,
        ):
            state = {}

            def stage_a(c):
                r0 = c * P
                xt = [
                    xp.tile([P, wtile], f32, tag="xt", name=f"xt{c}_{j}")
                    for j in range(ntiles)
                ]
                mx = sp.tile([P, ntiles], f32, tag="mx", name=f"mx{c}")
                mn = sp.tile([P, ntiles], f32, tag="mn", name=f"mn{c}")
                xmax = sp.tile([P, 1], f32, tag="xmax", name=f"xmax{c}")
                xmin = sp.tile([P, 1], f32, tag="xmin", name=f"xmin{c}")
                bias0 = sp.tile([P, 1], f32, tag="bias0", name=f"bias0{c}")
                for j in range(ntiles):
                    nc.sync.dma_start(
                        out=xt[j], in_=x[r0 : r0 + P, j * wtile : (j + 1) * wtile]
                    )
                for j in range(ntiles):
                    nc.vector.tensor_reduce(
                        out=mx[:, j : j + 1], in_=xt[j], axis=AX, op=ALU.max
                    )
                    nc.vector.tensor_reduce(
                        out=mn[:, j : j + 1], in_=xt[j], axis=AX, op=ALU.min
                    )
                nc.vector.tensor_reduce(out=xmax, in_=mx, axis=AX, op=ALU.max)
                nc.vector.tensor_reduce(out=xmin, in_=mn, axis=AX, op=ALU.min)
                # bias0 = 0.5*xmax + 1 - xmin
                nc.vector.tensor_scalar(
                    out=bias0,
                    in0=xmax,
                    scalar1=0.5,
                    scalar2=1.0,
                    op0=ALU.mult,
                    op1=ALU.add,
                )
                nc.vector.tensor_tensor(
                    out=bias0, in0=bias0, in1=xmin, op=ALU.subtract
                )
                state[c] = (xt, bias0)

            def stage_b(c):
                r0 = c * P
                xt, bias0 = state.pop(c)
                s = sp.tile([P, ntiles], f32, tag="s", name=f"s{c}")
                ssum = sp.tile([P, 1], f32, tag="ssum", name=f"ssum{c}")
                rcp = sp.tile([P, 1], f32, tag="rcp", name=f"rcp{c}")
                # w = (0.5*x + bias0)^2 in place, with per-row sum
                for j in range(ntiles):
                    nc.scalar.activation(
                        out=xt[j],
                        in_=xt[j],
                        func=ACTF.Square,
                        bias=bias0,
                        scale=0.5,
                        accum_out=s[:, j : j + 1],
                    )
                nc.vector.tensor_reduce(out=ssum, in_=s, axis=AX, op=ALU.add)
                nc.vector.tensor_scalar(
                    out=ssum, in0=ssum, scalar1=1e-12, scalar2=None, op0=ALU.add
                )
                nc.vector.reciprocal(out=rcp, in_=ssum)
                # out = w * (1/S) in place, then store
                for j in range(ntiles):
                    if j >= ntiles - DVE_SCALE_TILES:
                        nc.vector.tensor_scalar(
                            out=xt[j],
                            in0=xt[j],
                            scalar1=rcp,
                            scalar2=None,
                            op0=ALU.mult,
                        )
                    else:
                        nc.scalar.activation(
                            out=xt[j], in_=xt[j], func=ACTF.Copy, bias=0.0, scale=rcp
                        )
                    nc.sync.dma_start(
                        out=out[r0 : r0 + P, j * wtile : (j + 1) * wtile], in_=xt[j]
                    )

            for c in range(nchunks):
                stage_a(c)
                if c >= 1:
                    stage_b(c - 1)
            stage_b(nchunks - 1)
    # Run Bacc passes (register allocation + the 1-wait/inst sync split).
    # run_bass_via_pjrt serializes nc as-is and never finalizes prebuilt
    # modules; without this walrus crashes on unallocated virtual registers.
    nc.finalize()
    return nc


def kernel(x: np.ndarray) -> np.ndarray:
    from concourse.bass_utils import run_bass_kernel_spmd

    x = np.ascontiguousarray(x, dtype=np.float32)
    assert x.shape == (ROWS, COLS)
    nc = _build(RPC, COLS, WTILE)
    in_maps = [{"x": x[i * RPC : (i + 1) * RPC]} for i in range(N_CORES)]
    res = run_bass_kernel_spmd(nc, in_maps, list(range(N_CORES)))
    return np.concatenate([r["out"] for r in res.results], axis=0)
